# revision 1
# baseline (speedup 1.0000x reference)
"""Trainium2 Bass kernel for nn_CompletenessLoss (OHEM hinge loss with top-k).

Self-contained: accepts FULL inputs, shards over 8 NeuronCores internally
(data-parallel over the group dimension), returns the full scalar output.

Math (reference):
  scores[i]  = pred[i, labels[i]-1]
  groups of 64 rows: first 8 are "positive", last 56 are "negative"
  pos_ls = sum over all positive rows of relu(1 - s)
  neg_ls = sum over groups of (sum of top-9 of relu(1 + s) over 56 negatives)
  out    = (pos_ls + neg_ls) / (num_pos + int(num_neg * 0.17))

V2 gather strategy (per core, 32768 rows = 128 partitions x 256 rows):
  The label-indexed gather is split across two engines running in parallel:
  - rows t in [0, XG): GPSIMD ap_gather. Indices are shared per 16-partition
    group, so the host packs index lists where position 16k+q holds the
    index of partition (16g+q)'s row k; each partition's wanted value lands
    on the "diagonal" position 16k+(p%16). A static-per-input mask multiply
    + segmented reduce (DVE, cheap) extracts the diagonal.
  - rows t in [XG, 256): DVE scalar_tensor_tensor (iota==label)*pred with
    fused accumulate (the V1 path).
  Both write into one scores[P, 256] tile; phase 2 (hinge + top-9) as in V1.
"""

import numpy as np

# Problem geometry (hardcoded per the harness contract).
N_FULL = 262144
D = 200                      # pred_dim
GS = 64                      # sample_group_size
SS = 8                       # sample_split (positives per group)
OHEM_RATIO = 0.17
KEEP = int((GS - SS) * OHEM_RATIO)   # 9 hardest negatives kept per group

N_CORES = 8
ROWS = N_FULL // N_CORES     # 32768 rows per core
P = 128                      # SBUF partitions
NTILES = ROWS // P           # 256 rows per partition = 4 groups
CHUNK = 16                   # rows-per-partition per DMA/compute chunk
XG = 80                      # rows per partition gathered on GPSIMD
XS = NTILES - XG             # rows via host one-hot mask + DVE mult/reduce
XD = 0

_compiled = None             # cached program so repeat calls skip rebuild


def build_nc():
    """Build the per-core Bass program (SPMD across the 8 cores)."""
    import concourse.bacc as bacc
    import concourse.tile as tile
    from concourse import mybir

    f32 = mybir.dt.float32
    bf16 = mybir.dt.bfloat16
    i16 = mybir.dt.int16

    nc = bacc.Bacc("TRN2", target_bir_lowering=False, debug=False,
                   num_devices=N_CORES)
    pred_t = nc.dram_tensor("pred", [ROWS, D], bf16, kind="ExternalInput")
    # labt[p, t] = labels[p*NTILES + t] - 1, as f32 (stt scalar operand)
    lab_t = nc.dram_tensor("labt", [P, NTILES], f32, kind="ExternalInput")
    # cio: the 0..D-1 class ramp, broadcast-DMA'd to all partitions
    cio_t = nc.dram_tensor("cio", [1, D], bf16, kind="ExternalInput")
    # idx[p, t] = (t%16)*100 + lab>>1 for gpsimd rows t in [0, XG)
    idx_t = nc.dram_tensor("idx", [P, XG], i16, kind="ExternalInput")
    # msk[p, t*32 + q*2 + e] = (q == p%16) & (e == lab%2), bf16
    msk_t = nc.dram_tensor("msk", [P, XG * 32], bf16, kind="ExternalInput")
    # smask: host one-hot mask for the s-lane rows, streamed per chunk
    smask_t = nc.dram_tensor("smask", [P, XS * D], bf16, kind="ExternalInput")
    out_t = nc.dram_tensor("partial", [P, 2], f32, kind="ExternalOutput")

    with tile.TileContext(nc) as tc:
        _body(tc, pred_t.ap(), lab_t.ap(), cio_t.ap(), idx_t.ap(),
              msk_t.ap(), smask_t.ap(), out_t.ap())
    nc.compile()
    return nc


def _body(tc, pred, labt, cio, idx, msk, smask, out):
    from concourse import mybir
    import concourse.bass as bass
    from contextlib import ExitStack

    nc = tc.nc
    f32 = mybir.dt.float32
    bf16 = mybir.dt.bfloat16
    i16 = mybir.dt.int16
    AX = mybir.AxisListType
    OP = mybir.AluOpType
    AF = mybir.ActivationFunctionType

    with ExitStack() as ctx:
        singles = ctx.enter_context(tc.tile_pool(name="singles", bufs=1))
        ph2 = ctx.enter_context(tc.tile_pool(name="ph2", bufs=2))
        scr = ctx.enter_context(tc.tile_pool(name="scr", bufs=4))

        # --- one-time constants / inputs ---
        iota = singles.tile([P, D], bf16)
        nc.scalar.dma_start(
            out=iota,
            in_=bass.AP(tensor=cio.tensor, offset=cio.offset,
                        ap=[[0, P]] + list(cio.ap)))
        labs = singles.tile([P, NTILES], f32)
        nc.scalar.dma_start(out=labs, in_=labt)
        idxs = singles.tile([P, XG], i16)
        nc.scalar.dma_start(out=idxs, in_=idx)
        msks = singles.tile([P, XG, 32], bf16)
        nc.scalar.dma_start(out=msks, in_=msk)

        pred_sb = singles.tile([P, NTILES, D], bf16)
        out2 = singles.tile([P, XG * 16, 2], bf16)
        scores = singles.tile([P, NTILES], f32)

        # Warm DVE so hot-loop ops only wait on their pred-chunk DMA.
        warm = singles.tile([P, 1], f32)
        nc.vector.tensor_scalar(out=warm, in0=labs[:, 0:1], scalar1=0.0,
                                scalar2=1.0, op0=OP.mult, op1=OP.mult)
        warm2 = singles.tile([P, 1], bf16)
        nc.vector.tensor_copy(warm2, iota[:, 0:1])
        # Warm GPSIMD (pays the one-time ucode IRAM load) with a tiny
        # gather on the idx tile itself, before the first pred chunk lands.
        wz_idx = singles.tile([P, 16], i16)
        nc.gpsimd.memset(wz_idx, 0)
        warm3 = singles.tile([P, 16, 2], bf16)
        nc.gpsimd.ap_gather(out_ap=warm3, in_ap=iota.rearrange(
            "p (a b) -> p a b", b=2), idxs_ap=wz_idx[:, 0:1],
            channels=P, num_elems=D // 2, d=2, num_idxs=16)

        # --- phase 1: stream pred; gather on GPSIMD + DVE in parallel ---
        pred_v = pred.rearrange("(p t) j -> p t j", p=P)
        # gather chunks grow geometrically (amortize ~2.4us/op overhead);
        # DVE chunks stay small for pipelining. DMA order feeds both early.
        gchunks = [(i * CHUNK, CHUNK) for i in range(XG // CHUNK)]
        schunks = [(XG + i * CHUNK, CHUNK) for i in range(XS // CHUNK)]
        dchunks = [(XG + XS + i * CHUNK, CHUNK) for i in range(XD // CHUNK)]
        order = []
        gi, si, di = 0, 0, 0
        while gi < len(gchunks) or si < len(schunks) or di < len(dchunks):
            if gi < len(gchunks):
                order.append(("g", gchunks[gi])); gi += 1
            if si < len(schunks):
                order.append(("s", schunks[si])); si += 1
            if si < len(schunks):
                order.append(("s", schunks[si])); si += 1
            if di < len(dchunks):
                order.append(("d", dchunks[di])); di += 1

        for kind, (tb, csz) in order:
            nc.sync.dma_start(out=pred_sb[:, tb:tb + csz, :],
                              in_=pred_v[:, tb:tb + csz, :])
            if kind == "g":
                # gpsimd gather: shared indices per 16-partition group
                nc.gpsimd.ap_gather(
                    out_ap=out2[:, tb * 16:(tb + csz) * 16, :],
                    in_ap=pred_sb[:, tb:tb + csz, :].rearrange(
                        "p t (a b) -> p (t a) b", b=2),
                    idxs_ap=idxs[:, tb:tb + csz],
                    channels=P, num_elems=csz * (D // 2), d=2,
                    num_idxs=csz * 16)
            elif kind == "s":
                # host one-hot mask chunk arrives on the scalar DMA queue;
                # DVE does bulk 2x multiply (in place over pred) + seg reduce
                mk = scr.tile([P, CHUNK, D], bf16, tag="mk")
                sm_v = smask.rearrange("p (t j) -> p t j", j=D)
                nc.scalar.dma_start(out=mk[:, 0:csz, :],
                                     in_=sm_v[:, tb - XG:tb - XG + csz, :])
                pr = pred_sb[:, tb:tb + csz, :]
                nc.vector.tensor_tensor(out=pr, in0=pr, in1=mk[:, 0:csz, :],
                                        op=OP.mult)
                # masked rows are one-nonzero-among-zeros: bf16 pairwise
                # fold is exact and runs at 2x; the 1x reduce sees half
                nc.vector.tensor_tensor(
                    out=pr[:, :, 0:D // 2], in0=pr[:, :, 0:D // 2],
                    in1=pr[:, :, D // 2:D], op=OP.add)
                nc.vector.tensor_reduce(
                    out=scores[:, tb:tb + csz], in_=pr[:, :, 0:D // 2],
                    axis=AX.X, op=OP.add)
            else:
                for b in range(csz):
                    t = tb + b
                    nc.vector.scalar_tensor_tensor(
                        out=pred_sb[:, t, :], in0=iota,
                        scalar=labs[:, t:t + 1],
                        in1=pred_sb[:, t, :], op0=OP.is_equal, op1=OP.mult,
                        accum_out=scores[:, t:t + 1])

        # extracts AFTER all stt issues: the Vector queue is in-order, so an
        # extract waiting on a late gather must not block pending stt work.
        for tb, csz in gchunks:
            o2 = out2[:, tb * 16:(tb + csz) * 16, :]
            nc.vector.tensor_tensor(
                out=o2, in0=o2,
                in1=msks[:, tb:tb + csz, :].rearrange(
                    "p t (a b) -> p (t a) b", b=2),
                op=OP.mult)
            o3 = out2[:, tb * 16:(tb + csz) * 16, :].rearrange(
                "p (t a) b -> p t (a b)", a=16)
            nc.vector.tensor_tensor(
                out=o3[:, :, 0:16], in0=o3[:, :, 0:16], in1=o3[:, :, 16:32],
                op=OP.add)
            nc.vector.tensor_reduce(
                out=scores[:, tb:tb + csz], in_=o3[:, :, 0:16],
                axis=AX.X, op=OP.add)

        # --- phase 2: per partition, 4 whole groups along the free axis ---
        gpp = NTILES // GS
        pp = singles.tile([P, gpp], f32)             # pos sums per group
        negacc = singles.tile([P, 2 * gpp], f32)     # top8-sum & 9th cols
        for g in range(gpp):
            stg = scores[:, g * GS:(g + 1) * GS]
            ptmp = ph2.tile([P, SS], f32, tag="ptmp")
            nc.scalar.activation(
                out=ptmp, in_=stg[:, 0:SS], func=AF.Relu,
                bias=1.0, scale=-1.0, accum_out=pp[:, g:g + 1])
            nl = ph2.tile([P, GS - SS], f32, tag="nl")
            nc.scalar.activation(
                out=nl, in_=stg[:, SS:GS],
                func=AF.Relu, bias=1.0, scale=1.0)
            m8 = ph2.tile([P, 8], f32, tag="m8")
            nc.vector.max(out=m8, in_=nl)
            nc.vector.match_replace(
                out=nl, in_to_replace=m8, in_values=nl, imm_value=-1.0)
            nc.vector.tensor_reduce(
                out=negacc[:, 2 * g:2 * g + 1], in_=m8, axis=AX.X, op=OP.add)
            nc.vector.tensor_reduce(
                out=negacc[:, 2 * g + 1:2 * g + 2], in_=nl, axis=AX.X,
                op=OP.max)

        # --- final per-partition reduction -> [P, 2] ---
        res = singles.tile([P, 2], f32)
        nc.vector.tensor_reduce(out=res[:, 0:1], in_=pp, axis=AX.X, op=OP.add)
        nc.vector.tensor_reduce(out=res[:, 1:2], in_=negacc, axis=AX.X,
                                op=OP.add)
        nc.sync.dma_start(out=out, in_=res)


def _get_compiled():
    global _compiled
    if _compiled is None:
        _compiled = build_nc()
    return _compiled


def _prep_core_inputs(pred, labels):
    """Split full inputs into per-core input maps."""
    import ml_dtypes
    pred = np.asarray(pred).astype(ml_dtypes.bfloat16)
    lab = np.asarray(labels).astype(np.int64)
    cio = np.arange(D).reshape(1, D).astype(ml_dtypes.bfloat16)
    k16 = (np.arange(XG, dtype=np.int64) % CHUNK)[None, :]      # [1, XG]
    qsel = (np.arange(P, dtype=np.int64) % 16)                  # [P]
    in_maps = []
    for c in range(N_CORES):
        sl = slice(c * ROWS, (c + 1) * ROWS)
        lab_sh = (lab[sl] - 1).reshape(P, NTILES)                # int64
        labt = np.ascontiguousarray(lab_sh.astype(np.float32))
        lg = lab_sh[:, :XG]                                      # [P, XG]
        idxs = (k16 * (D // 2) + (lg >> 1)).astype(np.int16)
        # msk[p, t, q*2+e] = (q == p%16) & (e == lab%2)
        msk = np.zeros((P, XG, 32), dtype=ml_dtypes.bfloat16)
        e = (lg & 1).astype(np.int64)                            # [P, XG]
        pi = np.arange(P)[:, None]
        ti = np.arange(XG)[None, :]
        msk[pi, ti, qsel[:, None] * 2 + e] = 1
        ls = lab_sh[:, XG:]                                      # [P, XS]
        smask = np.zeros((P, XS, D), dtype=ml_dtypes.bfloat16)
        smask[np.arange(P)[:, None], np.arange(XS)[None, :], ls] = 1
        in_maps.append({
            "pred": np.ascontiguousarray(pred[sl]),
            "smask": np.ascontiguousarray(smask.reshape(P, XS * D)),
            "labt": labt,
            "cio": cio,
            "idx": np.ascontiguousarray(idxs),
            "msk": np.ascontiguousarray(msk.reshape(P, XG * 32)),
        })
    return in_maps


def _finalize(results):
    pos = 0.0
    neg = 0.0
    for r in results:
        part = r["partial"].astype(np.float64)
        pos += part[:, 0].sum()
        neg += part[:, 1].sum()
    num_pos = (N_FULL // GS) * SS
    num_neg = N_FULL - num_pos
    denom = float(num_pos + int(num_neg * OHEM_RATIO))
    return np.float32((pos + neg) / denom)


def kernel(pred, labels, sample_split, sample_group_size):
    assert int(sample_split) == SS and int(sample_group_size) == GS
    from concourse.bass_utils import run_bass_kernel_spmd

    nc = _get_compiled()
    in_maps = _prep_core_inputs(pred, labels)
    res = run_bass_kernel_spmd(nc, in_maps, core_ids=list(range(N_CORES)))
    return _finalize(res.results)

